# revision 6
# baseline (speedup 1.0000x reference)
"""Trainium2 Bass kernel for the additive-attention module.

reference:
    hidden = concat([adj, static, dynamic, broadcast(dec)], axis=1)   # [B, 4H, N]
    Wh     = tanh(einsum('hk,bkn->bhn', W[0], hidden))                # [B, H, N]
    attns  = einsum('h,bhn->bn', v[0,0], Wh)[:, None, :]              # [B, 1, N]
    out    = softmax(attns, axis=2)

Strategy (data-parallel over batch, 8 NeuronCores, 32 batches/core):
  - Split W[0] [H, 4H] into 4 HxH blocks. The dec block contributes a
    per-(b,h) bias (constant over n): bias = dec @ W4.T, computed on host
    (tiny). The three big blocks run as float32r matmuls on the PE array,
    accumulated in PSUM: Wh_pre[b] = W1@adj[b] + W2@static[b] + W3@dyn[b].
  - tanh(x + bias) fused on ScalarE (per-partition AP bias).
  - v-dot via PE: for batch j, lhsT = [128, 32] window of a zero-padded v
    buffer with v in column j -> matmul accumulates v.tanh(.) into row j of
    a [32, 500] PSUM scores tile. 32 such matmuls share one accumulation
    group per N-tile, so scores land batch-major in partitions with no
    cross-partition copies.
  - softmax on [32, 1000]: VectorE max (negated), ScalarE exp with bias=-max
    and accum_out running sum, VectorE reciprocal + tensor_scalar multiply.
  - Inputs are host-transposed to [H, B, N] so each per-core DMA group of
    G batches is one 128-partition transfer with 16 KiB contiguous
    per-partition chunks (near-peak HBM bandwidth).
"""

import sys

if "/opt/trn_rl_repo" not in sys.path:
    sys.path.insert(0, "/opt/trn_rl_repo")

from contextlib import ExitStack

import numpy as np

import concourse.tile as tile
from concourse import bacc, mybir
from concourse.bass_utils import run_bass_kernel_spmd

N_CORES = 8
B, H, N = 256, 128, 1000
BPC = B // N_CORES  # 32 batches per core
G = 4               # batches per DMA group
NTS = 500           # free-dim tile size (PSUM bank limit: 512 fp32)
NT = N // NTS
F32 = mybir.dt.float32
F32R = mybir.dt.float32r

_NC_CACHE = {}


def _build():
    nc = bacc.Bacc("TRN2", target_bir_lowering=False, debug=False, num_devices=N_CORES)
    adj = nc.dram_tensor("adj", [H, BPC, N], F32R, kind="ExternalInput").ap()
    sta = nc.dram_tensor("sta", [H, BPC, N], F32R, kind="ExternalInput").ap()
    dyn = nc.dram_tensor("dyn", [H, BPC, N], F32R, kind="ExternalInput").ap()
    wt = nc.dram_tensor("wt", [H, 3 * H], F32R, kind="ExternalInput").ap()
    vpad = nc.dram_tensor("vpad", [H, 2 * BPC - 1], F32R, kind="ExternalInput").ap()
    bias = nc.dram_tensor("bias", [H, BPC], F32, kind="ExternalInput").ap()
    out = nc.dram_tensor("out", [BPC, N], F32, kind="ExternalOutput").ap()

    with tile.TileContext(nc) as tc, ExitStack() as ctx:
        consts = ctx.enter_context(tc.tile_pool(name="consts", bufs=1))
        inp = ctx.enter_context(tc.tile_pool(name="inp", bufs=3))
        acts = ctx.enter_context(tc.tile_pool(name="acts", bufs=3))
        pwh = ctx.enter_context(tc.tile_pool(name="pwh", bufs=2, space="PSUM"))
        psc = ctx.enter_context(tc.tile_pool(name="psc", bufs=1, space="PSUM"))
        smax = ctx.enter_context(tc.tile_pool(name="smax", bufs=1))

        wt_sb = consts.tile([H, 3 * H], F32R, tag="wt")
        vpad_sb = consts.tile([H, 2 * BPC - 1], F32R, tag="vpad")
        bias_sb = consts.tile([H, BPC], F32, tag="bias")

        def load_consts():
            nc.scalar.dma_start(out=wt_sb[:], in_=wt[:])
            nc.scalar.dma_start(out=vpad_sb[:], in_=vpad[:])
            nc.scalar.dma_start(out=bias_sb[:], in_=bias[:])

        # two K=64 row-tiled partial v-dots per tile, in separate PSUM banks;
        # they run concurrently on disjoint 64-row halves of the PE array, so
        # the v-dot costs ~250 streaming cycles instead of 500.
        sc = [
            [psc.tile([BPC, NTS], F32, tag=f"sc{t}_{i}", name=f"sc{t}_{i}") for i in range(2)]
            for t in range(NT)
        ]

        def vdot(b, t, th):
            # accumulate v[half] . tanh(Wh[b])[half] into row b of sc[t][half]
            for i in range(2):
                nc.tensor.matmul(
                    sc[t][i][:],
                    lhsT=vpad_sb[i * 64 : (i + 1) * 64, BPC - 1 - b : 2 * BPC - 1 - b],
                    rhs=th[i * 64 : (i + 1) * 64, :],
                    start=(b == 0),
                    stop=(b == BPC - 1),
                    skip_group_check=True,
                    tile_position=(i * 64, 0),
                )

        pending = None  # one-deep software pipeline: PE runs vdot(i-1) after mm3(i)
        for g in range(BPC // G):
            at = inp.tile([H, G, N], F32R, tag="adj")
            nc.sync.dma_start(out=at[:], in_=adj[:, g * G : (g + 1) * G, :])
            st = inp.tile([H, G, N], F32R, tag="sta")
            nc.sync.dma_start(out=st[:], in_=sta[:, g * G : (g + 1) * G, :])
            dt = inp.tile([H, G, N], F32R, tag="dyn")
            nc.sync.dma_start(out=dt[:], in_=dyn[:, g * G : (g + 1) * G, :])
            if g == 0:
                load_consts()
            for j in range(G):
                b = g * G + j
                for t in range(NT):
                    s0 = t * NTS
                    pw = pwh.tile([H, NTS], F32, tag="pw")
                    for ti, src in enumerate((at, st, dt)):
                        nc.tensor.matmul(
                            pw[:],
                            lhsT=wt_sb[:, ti * H : (ti + 1) * H],
                            rhs=src[:, j, s0 : s0 + NTS],
                            start=(ti == 0),
                            stop=(ti == 2),
                            skip_group_check=True,
                        )
                    th = acts.tile([H, NTS], F32R, tag="th")
                    nc.scalar.activation(
                        th[:],
                        pw[:],
                        mybir.ActivationFunctionType.Tanh,
                        bias=bias_sb[:, b : b + 1],
                    )
                    if pending is not None:
                        vdot(*pending)
                    pending = (b, t, th)
        vdot(*pending)

        ssb = smax.tile([BPC, N], F32, tag="ssb")
        for t in range(NT):
            nc.vector.tensor_copy(ssb[:, t * NTS : (t + 1) * NTS], sc[t][0][:])
            nc.vector.tensor_add(
                ssb[:, t * NTS : (t + 1) * NTS],
                ssb[:, t * NTS : (t + 1) * NTS],
                sc[t][1][:],
            )
        negmax = smax.tile([BPC, 1], F32, tag="negmax")
        nc.vector.tensor_reduce(
            out=negmax[:], in_=ssb[:], axis=mybir.AxisListType.X,
            op=mybir.AluOpType.max, negate=True,
        )
        esb = smax.tile([BPC, N], F32, tag="esb")
        sums = smax.tile([BPC, 1], F32, tag="sums")
        nc.scalar.activation(
            esb[:], ssb[:], mybir.ActivationFunctionType.Exp,
            bias=negmax[:], accum_out=sums[:],
        )
        rcp = smax.tile([BPC, 1], F32, tag="rcp")
        nc.vector.reciprocal(rcp[:], sums[:])
        osb = smax.tile([BPC, N], F32, tag="osb")
        nc.vector.tensor_scalar_mul(osb[:], esb[:], rcp[:])
        nc.sync.dma_start(out=out[:], in_=osb[:])

    nc.compile()
    return nc


def _get_nc():
    if "nc" not in _NC_CACHE:
        _NC_CACHE["nc"] = _build()
    return _NC_CACHE["nc"]


def _prep_in_maps(adj_hidden, static_hidden, dynamic_hidden, decoder_hidden, v, W):
    f32 = lambda x: np.asarray(x, dtype=np.float32)
    # [B, H, N] -> [H, B, N] so per-core DMA groups are contiguous per partition
    adj_t = np.ascontiguousarray(f32(adj_hidden).transpose(1, 0, 2))
    sta_t = np.ascontiguousarray(f32(static_hidden).transpose(1, 0, 2))
    dyn_t = np.ascontiguousarray(f32(dynamic_hidden).transpose(1, 0, 2))
    W0 = f32(W)[0]  # [H, 4H]
    # wt[k, i*H + h] = W0[h, i*H + k] : block i is the lhsT of W-block i
    wt_host = np.ascontiguousarray(
        W0[:, : 3 * H].reshape(H, 3, H).transpose(2, 1, 0).reshape(H, 3 * H)
    )
    vv = f32(v).reshape(H)
    vpad_host = np.zeros((H, 2 * BPC - 1), np.float32)
    vpad_host[:, BPC - 1] = vv
    dec = f32(decoder_hidden)  # [B, H]
    bias_all = dec @ W0[:, 3 * H :].T  # [B, H]

    in_maps = []
    for c in range(N_CORES):
        lo, hi = c * BPC, (c + 1) * BPC
        in_maps.append(
            {
                "adj": np.ascontiguousarray(adj_t[:, lo:hi, :]),
                "sta": np.ascontiguousarray(sta_t[:, lo:hi, :]),
                "dyn": np.ascontiguousarray(dyn_t[:, lo:hi, :]),
                "wt": wt_host,
                "vpad": vpad_host,
                "bias": np.ascontiguousarray(bias_all[lo:hi, :].T),
            }
        )
    return in_maps


def _run(in_maps, trace=False, **kw):
    nc = _get_nc()
    res = run_bass_kernel_spmd(nc, in_maps, core_ids=list(range(N_CORES)), trace=trace, **kw)
    full = np.concatenate(
        [res.results[c]["out"][:, None, :] for c in range(N_CORES)], axis=0
    )
    return full, res


def kernel(adj_hidden, static_hidden, dynamic_hidden, decoder_hidden, v, W):
    in_maps = _prep_in_maps(adj_hidden, static_hidden, dynamic_hidden, decoder_hidden, v, W)
    full, _ = _run(in_maps, trace=False)
    return full


# revision 7
# speedup vs baseline: 1.1012x; 1.1012x over previous
"""Trainium2 Bass kernel for the additive-attention module.

reference:
    hidden = concat([adj, static, dynamic, broadcast(dec)], axis=1)   # [B, 4H, N]
    Wh     = tanh(einsum('hk,bkn->bhn', W[0], hidden))                # [B, H, N]
    attns  = einsum('h,bhn->bn', v[0,0], Wh)[:, None, :]              # [B, 1, N]
    out    = softmax(attns, axis=2)

Strategy (data-parallel over batch, 8 NeuronCores, 32 batches/core):
  - Split W[0] [H, 4H] into 4 HxH blocks. The dec block contributes a
    per-(b,h) bias (constant over n): bias = dec @ W4.T, computed on host
    (tiny). The three big blocks run as float32r matmuls on the PE array,
    accumulated in PSUM: Wh_pre[b] = W1@adj[b] + W2@static[b] + W3@dyn[b].
  - tanh(x + bias) fused on ScalarE (per-partition AP bias).
  - v-dot via PE: for batch j, lhsT = [128, 32] window of a zero-padded v
    buffer with v in column j -> matmul accumulates v.tanh(.) into row j of
    a [32, 500] PSUM scores tile. 32 such matmuls share one accumulation
    group per N-tile, so scores land batch-major in partitions with no
    cross-partition copies.
  - softmax on [32, 1000]: VectorE max (negated), ScalarE exp with bias=-max
    and accum_out running sum, VectorE reciprocal + tensor_scalar multiply.
  - Inputs are host-transposed to [H, B, N] so each per-core DMA group of
    G batches is one 128-partition transfer with 16 KiB contiguous
    per-partition chunks (near-peak HBM bandwidth).
"""

import sys

if "/opt/trn_rl_repo" not in sys.path:
    sys.path.insert(0, "/opt/trn_rl_repo")

from contextlib import ExitStack

import numpy as np

import concourse.tile as tile
from concourse import bacc, mybir
from concourse.bass_utils import run_bass_kernel_spmd

N_CORES = 8
B, H, N = 256, 128, 1000
BPC = B // N_CORES  # 32 batches per core
G = 8               # batches per DMA group
NTS = 500           # free-dim tile size (PSUM bank limit: 512 fp32)
NT = N // NTS
F32 = mybir.dt.float32
F32R = mybir.dt.float32r

_NC_CACHE = {}


def _build():
    nc = bacc.Bacc("TRN2", target_bir_lowering=False, debug=False, num_devices=N_CORES)
    adj = nc.dram_tensor("adj", [H, BPC, N], F32R, kind="ExternalInput").ap()
    sta = nc.dram_tensor("sta", [H, BPC, N], F32R, kind="ExternalInput").ap()
    dyn = nc.dram_tensor("dyn", [H, BPC, N], F32R, kind="ExternalInput").ap()
    wt = nc.dram_tensor("wt", [H, 3 * H], F32R, kind="ExternalInput").ap()
    vpad = nc.dram_tensor("vpad", [H, 2 * BPC - 1], F32R, kind="ExternalInput").ap()
    bias = nc.dram_tensor("bias", [H, BPC], F32, kind="ExternalInput").ap()
    out = nc.dram_tensor("out", [BPC, N], F32, kind="ExternalOutput").ap()

    with tile.TileContext(nc) as tc, ExitStack() as ctx:
        consts = ctx.enter_context(tc.tile_pool(name="consts", bufs=1))
        inp = ctx.enter_context(tc.tile_pool(name="inp", bufs=2))
        acts = ctx.enter_context(tc.tile_pool(name="acts", bufs=2))
        pwh = ctx.enter_context(tc.tile_pool(name="pwh", bufs=2, space="PSUM"))
        psc = ctx.enter_context(tc.tile_pool(name="psc", bufs=1, space="PSUM"))
        smax = ctx.enter_context(tc.tile_pool(name="smax", bufs=1))

        wt_sb = consts.tile([H, 3 * H], F32R, tag="wt")
        vpad_sb = consts.tile([H, 2 * BPC - 1], F32R, tag="vpad")
        bias_sb = consts.tile([H, BPC], F32, tag="bias")

        def load_consts():
            nc.scalar.dma_start(out=wt_sb[:], in_=wt[:])
            nc.scalar.dma_start(out=vpad_sb[:], in_=vpad[:])
            nc.scalar.dma_start(out=bias_sb[:], in_=bias[:])

        # two K=64 row-tiled partial v-dots per tile, in separate PSUM banks;
        # they run concurrently on disjoint 64-row halves of the PE array, so
        # the v-dot costs ~250 streaming cycles instead of 500.
        sc = [
            [psc.tile([BPC, NTS], F32, tag=f"sc{t}_{i}", name=f"sc{t}_{i}") for i in range(2)]
            for t in range(NT)
        ]

        def vdot(b, t, th):
            # accumulate v[half] . tanh(Wh[b])[half] into row b of sc[t][half]
            for i in range(2):
                nc.tensor.matmul(
                    sc[t][i][:],
                    lhsT=vpad_sb[i * 64 : (i + 1) * 64, BPC - 1 - b : 2 * BPC - 1 - b],
                    rhs=th[i * 64 : (i + 1) * 64, :],
                    start=(b == 0),
                    stop=(b == BPC - 1),
                    skip_group_check=True,
                    tile_position=(i * 64, 0),
                )

        pending = None  # one-deep software pipeline: PE runs vdot(i-1) after mm3(i)
        for g in range(BPC // G):
            at = inp.tile([H, G, N], F32R, tag="adj")
            nc.sync.dma_start(out=at[:], in_=adj[:, g * G : (g + 1) * G, :])
            st = inp.tile([H, G, N], F32R, tag="sta")
            nc.sync.dma_start(out=st[:], in_=sta[:, g * G : (g + 1) * G, :])
            dt = inp.tile([H, G, N], F32R, tag="dyn")
            nc.sync.dma_start(out=dt[:], in_=dyn[:, g * G : (g + 1) * G, :])
            if g == 0:
                load_consts()
            for j in range(G):
                b = g * G + j
                for t in range(NT):
                    s0 = t * NTS
                    pw = pwh.tile([H, NTS], F32, tag="pw")
                    for ti, src in enumerate((at, st, dt)):
                        nc.tensor.matmul(
                            pw[:],
                            lhsT=wt_sb[:, ti * H : (ti + 1) * H],
                            rhs=src[:, j, s0 : s0 + NTS],
                            start=(ti == 0),
                            stop=(ti == 2),
                            skip_group_check=True,
                        )
                    th = acts.tile([H, NTS], F32R, tag="th")
                    nc.scalar.activation(
                        th[:],
                        pw[:],
                        mybir.ActivationFunctionType.Tanh,
                        bias=bias_sb[:, b : b + 1],
                    )
                    if pending is not None:
                        vdot(*pending)
                    pending = (b, t, th)
        vdot(*pending)

        ssb = smax.tile([BPC, N], F32, tag="ssb")
        for t in range(NT):
            nc.vector.tensor_copy(ssb[:, t * NTS : (t + 1) * NTS], sc[t][0][:])
            nc.vector.tensor_add(
                ssb[:, t * NTS : (t + 1) * NTS],
                ssb[:, t * NTS : (t + 1) * NTS],
                sc[t][1][:],
            )
        negmax = smax.tile([BPC, 1], F32, tag="negmax")
        nc.vector.tensor_reduce(
            out=negmax[:], in_=ssb[:], axis=mybir.AxisListType.X,
            op=mybir.AluOpType.max, negate=True,
        )
        esb = smax.tile([BPC, N], F32, tag="esb")
        sums = smax.tile([BPC, 1], F32, tag="sums")
        nc.scalar.activation(
            esb[:], ssb[:], mybir.ActivationFunctionType.Exp,
            bias=negmax[:], accum_out=sums[:],
        )
        rcp = smax.tile([BPC, 1], F32, tag="rcp")
        nc.vector.reciprocal(rcp[:], sums[:])
        nc.vector.tensor_scalar_mul(esb[:], esb[:], rcp[:])
        nc.sync.dma_start(out=out[:], in_=esb[:])

    nc.compile()
    return nc


def _get_nc():
    if "nc" not in _NC_CACHE:
        _NC_CACHE["nc"] = _build()
    return _NC_CACHE["nc"]


def _prep_in_maps(adj_hidden, static_hidden, dynamic_hidden, decoder_hidden, v, W):
    f32 = lambda x: np.asarray(x, dtype=np.float32)
    # [B, H, N] -> [H, B, N] so per-core DMA groups are contiguous per partition
    adj_t = np.ascontiguousarray(f32(adj_hidden).transpose(1, 0, 2))
    sta_t = np.ascontiguousarray(f32(static_hidden).transpose(1, 0, 2))
    dyn_t = np.ascontiguousarray(f32(dynamic_hidden).transpose(1, 0, 2))
    W0 = f32(W)[0]  # [H, 4H]
    # wt[k, i*H + h] = W0[h, i*H + k] : block i is the lhsT of W-block i
    wt_host = np.ascontiguousarray(
        W0[:, : 3 * H].reshape(H, 3, H).transpose(2, 1, 0).reshape(H, 3 * H)
    )
    vv = f32(v).reshape(H)
    vpad_host = np.zeros((H, 2 * BPC - 1), np.float32)
    vpad_host[:, BPC - 1] = vv
    dec = f32(decoder_hidden)  # [B, H]
    bias_all = dec @ W0[:, 3 * H :].T  # [B, H]

    in_maps = []
    for c in range(N_CORES):
        lo, hi = c * BPC, (c + 1) * BPC
        in_maps.append(
            {
                "adj": np.ascontiguousarray(adj_t[:, lo:hi, :]),
                "sta": np.ascontiguousarray(sta_t[:, lo:hi, :]),
                "dyn": np.ascontiguousarray(dyn_t[:, lo:hi, :]),
                "wt": wt_host,
                "vpad": vpad_host,
                "bias": np.ascontiguousarray(bias_all[lo:hi, :].T),
            }
        )
    return in_maps


def _run(in_maps, trace=False, **kw):
    nc = _get_nc()
    res = run_bass_kernel_spmd(nc, in_maps, core_ids=list(range(N_CORES)), trace=trace, **kw)
    full = np.concatenate(
        [res.results[c]["out"][:, None, :] for c in range(N_CORES)], axis=0
    )
    return full, res


def kernel(adj_hidden, static_hidden, dynamic_hidden, decoder_hidden, v, W):
    in_maps = _prep_in_maps(adj_hidden, static_hidden, dynamic_hidden, decoder_hidden, v, W)
    full, _ = _run(in_maps, trace=False)
    return full
